# revision 2
# baseline (speedup 1.0000x reference)
"""Trainium2 kernel for nn_ContrasiveLoss (segment-reduce contrastive loss).

Strategy (data-parallel, one image per NeuronCore, 8 cores):
  Per-image loss needs only per-segment statistics:
      counts[k], sums[k, c], s2[k] = sum_{n in k} ||f_n||^2
  (the variance term telescopes).  Statistics come from one-hot matmuls on
  the TensorEngine: pixels ride the contraction axis; a block-diagonal
  one-hot (8 pixel groups x 16 labels = 128 columns -> full-width stationary,
  which triggers the compiler's fast-weight-load) is the stationary operand.
  Two moving streams per 128-pixel window:
      A: features [128, 256]            -> psum_a [128, 256]
      B: [d2-per-group (8) | ones (1)]  -> psum_b [128, 9]
  where d2[p, w, g] = sum_c f^2 is produced by an elementwise square
  (split DVE/ACT) and a single fused innermost-axis tensor_reduce.
  Features are marshaled on host into a pixel-on-partition layout so the
  HBM->SBUF DMA is straight and contiguous (no xbar transpose).
  A tiny epilogue turns stats into the scalar loss; host sums 8 scalars.
"""

import numpy as np

import concourse.bass as bass
import concourse.mybir as mybir
import concourse.tile as tile
from concourse.bass_utils import run_bass_kernel_spmd
from concourse.vector_clock import ScopedClock

# ---------------------------------------------------------------- problem dims
B, C, H, W = 8, 32, 512, 512
K = 16
G = 8                    # pixel groups packed alongside channels (8*16 = 128)
N = H * W                # pixels per image
TW = 128                 # pixels (per group) contracted per matmul window
NW = N // (G * TW)       # 256 windows
CW = 16                  # windows per DMA chunk
NCHUNK = NW // CW        # 16 chunks
FCOLS = G * C            # 256 moving feature cols per window
D2S = 10                 # d2 slot stride (8 d2 + 1 ones + 1 pad)

DD = 2.5
GAMMA = 0.005

FP16 = mybir.dt.float16
FP32 = mybir.dt.float32

TRACE = False            # test harness flips this for NTFF profiling
SQ_ACT_FRAC = 0.78       # fraction of the squares computed on ScalarE
SQ_SPLIT = 2             # ops per engine per chunk for the squares

# ------------------------------------------------- container-specific patches
def _patch_tile_drain() -> None:
    """This container's walrus build accepts only ONE sync-wait command per
    instruction, but TileContext's tail drain attaches one wait per active
    semaphore lane.  Split the tail drain into a chain of single-wait drains.
    """
    if getattr(tile.TileContext, "_drain_split_patched", False):
        return

    def _drain_and_barrier(self, tick_clock, wait_clock):
        drain_inst = self.nc.sync.drain()
        wait_clock.add_sem_waits(
            drain_inst.ins, ScopedClock({None: tick_clock.global_clock})
        )
        si = drain_inst.ins.sync_info
        if si is not None and len(si.on_wait) > 1:
            waits = list(si.on_wait)
            drain_inst.ins.sync_info = mybir.SyncInfo(
                on_wait=[waits[0]], on_update=list(si.on_update)
            )
            for w in waits[1:]:
                d2 = self.nc.sync.drain()
                d2.ins.sync_info = mybir.SyncInfo(on_wait=[w], on_update=[])

        self.nc.all_engine_barrier()
        assert self.sems is not None
        popped = self.nc._tile_sem_poison_stack.pop()
        assert popped is self._sem_poison
        self.nc.clear_and_free_semaphores(list(self.sems.allocated().values()))
        self.nc.all_engine_barrier()

    tile.TileContext._drain_and_barrier = _drain_and_barrier
    tile.TileContext._drain_split_patched = True


def _split_multi_waits(nc) -> None:
    """Walrus accepts one sync-wait per instruction: hoist extra waits onto
    single-wait Drain instructions on the same engine, inserted just before."""
    for fn in nc.m.functions:
        for blk in fn.blocks:
            changed = False
            out = []
            for ins in blk.instructions:
                si = ins.sync_info
                if si is not None and len(si.on_wait) > 1:
                    changed = True
                    waits = list(si.on_wait)
                    for j, w in enumerate(waits[:-1]):
                        d = mybir.InstDrain(name=f"{ins.name}-ws{j}")
                        d.engine = ins.engine
                        d.sync_info = mybir.SyncInfo(on_wait=[w], on_update=[])
                        out.append(d)
                    ins.sync_info = mybir.SyncInfo(
                        on_wait=[waits[-1]], on_update=list(si.on_update)
                    )
                out.append(ins)
            if changed:
                blk.instructions = out


# ------------------------------------------------------------- device program
def _host_constants():
    # iota over one-hot columns j=(g,k): value = j % 16
    iota = np.broadcast_to(
        np.tile(np.arange(K, dtype=np.float16), G), (128, 128)
    ).copy()

    # stats layout: [A (256) | B (9)]; row r = (g, k)
    mask = np.zeros((128, FCOLS + 9), dtype=np.float32)
    for r in range(128):
        g = r // K
        mask[r, g * C:(g + 1) * C] = 1.0        # sums block of own group
        mask[r, FCOLS + g] = 1.0                # d2 of own group
        mask[r, FCOLS + 8] = 1.0                # counts column
    sel = np.zeros((128, K), dtype=np.float32)
    for r in range(128):
        sel[r, r % K] = 1.0
    ident16 = np.eye(16, dtype=np.float32)
    ones_row = np.ones((1, 16), dtype=np.float32)
    ones_col = np.ones((16, 1), dtype=np.float32)
    triu = np.triu(np.ones((K, K), dtype=np.float32), k=1)
    return iota, mask, sel, ident16, ones_row, ones_col, triu


def _build_kernel():
    _patch_tile_drain()
    nc = bass.Bass("TRN2")

    fpk = nc.dram_tensor("fpk", [NCHUNK * 128, CW * FCOLS], FP16,
                         kind="ExternalInput")
    labt = nc.dram_tensor("labt", [128, NW * G], FP16, kind="ExternalInput")
    out = nc.dram_tensor("out", [1, 1], FP32, kind="ExternalOutput")

    iota_np, mask_np, sel_np, id16_np, ones_row_np, ones_col_np, triu_np = \
        _host_constants()
    c_iota = nc.inline_tensor(iota_np, name="c_iota")
    c_mask = nc.inline_tensor(mask_np, name="c_mask")
    c_sel = nc.inline_tensor(sel_np, name="c_sel")
    c_id16 = nc.inline_tensor(id16_np, name="c_id16")
    c_ones_row = nc.inline_tensor(ones_row_np, name="c_ones_row")
    c_ones_col = nc.inline_tensor(ones_col_np, name="c_ones_col")
    c_triu = nc.inline_tensor(triu_np, name="c_triu")

    with tile.TileContext(nc) as tc:
        with (
            tc.tile_pool(name="consts", bufs=1) as consts,
            tc.tile_pool(name="oh", bufs=1) as ohp,
            tc.tile_pool(name="feat", bufs=3) as featp,
            tc.tile_pool(name="sq", bufs=3) as sqp,
            tc.tile_pool(name="d2", bufs=3) as d2p,
            tc.tile_pool(name="acc", bufs=1, space="PSUM") as accp,
            tc.tile_pool(name="eps", bufs=1, space="PSUM") as epsp,
            tc.tile_pool(name="epi", bufs=1) as epi,
        ):
            # ---- constants + labels into SBUF (scalar-engine DMA ring keeps
            # the sync ring free for the feature stream)
            sb_iota = consts.tile([128, 128], FP16)
            nc.scalar.dma_start(out=sb_iota, in_=c_iota[:, :])
            sb_mask = consts.tile([128, FCOLS + 9], FP32)
            nc.scalar.dma_start(out=sb_mask, in_=c_mask[:, :])
            sb_sel = consts.tile([128, K], FP32)
            nc.scalar.dma_start(out=sb_sel, in_=c_sel[:, :])
            sb_id16 = consts.tile([16, 16], FP32)
            nc.scalar.dma_start(out=sb_id16, in_=c_id16[:, :])
            sb_ones_row = consts.tile([1, 16], FP32)
            nc.scalar.dma_start(out=sb_ones_row, in_=c_ones_row[:, :])
            sb_ones_col = consts.tile([16, 1], FP32)
            nc.scalar.dma_start(out=sb_ones_col, in_=c_ones_col[:, :])
            sb_triu = consts.tile([16, 16], FP32)
            nc.scalar.dma_start(out=sb_triu, in_=c_triu[:, :])
            sb_labt = consts.tile([128, NW * G], FP16)
            nc.sync.dma_start(out=sb_labt, in_=labt[:, :])

            oh_full = ohp.tile([128, NW * 128], FP16)
            oh3 = oh_full.rearrange("p (w j) -> p w j", j=128)

            psum_a = accp.tile([128, FCOLS], FP32)
            psum_b = accp.tile([128, 9], FP32)

            ft_tiles = []
            for ci in range(NCHUNK):
                # ---- feature chunk: straight contiguous 1MB DMA
                ft = featp.tile([128, CW * FCOLS], FP16)
                nc.sync.dma_start(
                    out=ft, in_=fpk[ci * 128:(ci + 1) * 128, :]
                )
                ft_tiles.append(ft)

                # ---- one-hot for this chunk's windows (DVE)
                lab_b = bass.AP(
                    tensor=sb_labt.tensor,
                    offset=ci * CW * G,
                    ap=[[NW * G, 128], [G, CW], [1, G], [0, K]],
                )
                iota_b = bass.AP(
                    tensor=sb_iota.tensor,
                    offset=0,
                    ap=[[128, 128], [0, CW], [1, 128]],
                )
                nc.vector.tensor_tensor(
                    out=oh3[:, ci * CW:(ci + 1) * CW, :], in0=lab_b,
                    in1=iota_b, op=mybir.AluOpType.is_equal,
                )

                # ---- squares (ACT takes SQ_ACT_FRAC, DVE the rest)
                sqf = sqp.tile([128, CW * FCOLS], FP16)
                ncols = CW * FCOLS
                split = int(ncols * (1.0 - SQ_ACT_FRAC)) // 4 * 4
                dve_step = split // SQ_SPLIT
                act_step = (ncols - split) // SQ_SPLIT
                for si in range(SQ_SPLIT):
                    a = si * dve_step
                    b = (si + 1) * dve_step if si < SQ_SPLIT - 1 else split
                    if b > a:
                        nc.vector.tensor_mul(
                            sqf[:, a:b], ft[:, a:b], ft[:, a:b]
                        )
                    a = split + si * act_step
                    b = (split + (si + 1) * act_step
                         if si < SQ_SPLIT - 1 else ncols)
                    if b > a:
                        nc.scalar.activation(
                            out=sqf[:, a:b], in_=ft[:, a:b],
                            func=mybir.ActivationFunctionType.Square,
                        )

                # ---- d2 per (window, group): one fused innermost reduce
                d2t = d2p.tile([128, CW, D2S], FP16)
                sq4 = sqf.rearrange("p (w g c) -> p w g c", g=G, c=C)
                with nc.allow_low_precision("d2 fp16: 32-term sums, psum fp32"):
                    nc.vector.tensor_reduce(
                        out=d2t[:, :, 0:G], in_=sq4,
                        axis=mybir.AxisListType.X, op=mybir.AluOpType.add,
                    )
                nc.vector.memset(d2t[:, :, G:G + 1], 1.0)

                # ---- segment matmuls (one-hot stationary, 128 cols -> FWL)
                for wl in range(CW):
                    w = ci * CW + wl
                    lhsT = oh3[:, w, :]
                    nc.tensor.matmul(
                        psum_a[:, :], lhsT,
                        ft[:, wl * FCOLS:(wl + 1) * FCOLS],
                        start=(w == 0), stop=(w == NW - 1),
                    )
                    nc.tensor.matmul(
                        psum_b[:, :], lhsT, d2t[:, wl, 0:9],
                        start=(w == 0), stop=(w == NW - 1),
                    )

            # ================= epilogue: stats -> scalar loss =================
            stats = epi.tile([128, FCOLS + 9], FP32)
            nc.vector.tensor_copy(stats[:, 0:FCOLS], psum_a)
            nc.vector.tensor_copy(stats[:, FCOLS:FCOLS + 9], psum_b)

            masked = epi.tile([128, FCOLS + 9], FP32)
            nc.vector.tensor_mul(masked, stats, sb_mask)

            psum2 = epsp.tile([16, FCOLS + 9], FP32)
            nc.tensor.matmul(psum2[:, :], sb_sel, masked, start=True, stop=True)
            comb = epi.tile([16, FCOLS + 9], FP32)
            nc.vector.tensor_copy(comb, psum2)

            # sums over the 8 groups: innermost-g view [16, c:32, g:8]
            sums = epi.tile([16, C], FP32)
            comb_gview = bass.AP(
                tensor=comb.tensor, offset=0,
                ap=[[FCOLS + 9, 16], [1, C], [C, G]],
            )
            nc.vector.tensor_reduce(
                out=sums, in_=comb_gview, axis=mybir.AxisListType.X,
                op=mybir.AluOpType.add,
            )
            s2 = epi.tile([16, 1], FP32)
            nc.vector.tensor_reduce(
                out=s2, in_=comb[:, FCOLS:FCOLS + 8],
                axis=mybir.AxisListType.X, op=mybir.AluOpType.add,
            )
            counts = epi.tile([16, 1], FP32)
            nc.vector.tensor_copy(counts, comb[:, FCOLS + 8:FCOLS + 9])
            recip = epi.tile([16, 1], FP32)
            nc.vector.reciprocal(out=recip, in_=counts)

            means = epi.tile([16, C], FP32)
            nc.vector.tensor_scalar_mul(out=means, in0=sums, scalar1=recip)
            msq = epi.tile([16, C], FP32)
            nc.vector.tensor_mul(msq, means, means)
            m2 = epi.tile([16, 1], FP32)
            nc.vector.tensor_reduce(
                out=m2, in_=msq, axis=mybir.AxisListType.X,
                op=mybir.AluOpType.add,
            )
            vark = epi.tile([16, 1], FP32)
            nc.vector.tensor_scalar_mul(out=vark, in0=s2, scalar1=recip)
            nc.vector.tensor_sub(vark, vark, m2)

            # pairwise distances: diff2 = m2_i + m2_j - 2 * means @ means.T
            psumT = epsp.tile([32, 16], FP32)
            nc.tensor.transpose(psumT[:, :], means, sb_id16)
            meansT = epi.tile([32, 16], FP32)
            nc.vector.tensor_copy(meansT, psumT)
            meansTn2 = epi.tile([32, 16], FP32)
            nc.vector.tensor_scalar_mul(out=meansTn2, in0=meansT, scalar1=-2.0)

            psumR = epsp.tile([1, 16], FP32)
            nc.tensor.transpose(psumR[:, :], m2, sb_id16)
            m2row = epi.tile([1, 16], FP32)
            nc.vector.tensor_copy(m2row, psumR)

            psumD = epsp.tile([16, 16], FP32)
            nc.tensor.matmul(psumD[:, :], sb_ones_row, m2row,
                             start=True, stop=False)
            nc.tensor.matmul(psumD[:, :], m2row, sb_ones_row,
                             start=False, stop=False)
            nc.tensor.matmul(psumD[:, :], meansTn2, meansT,
                             start=False, stop=True)

            diff2 = epi.tile([16, 16], FP32)
            nc.vector.tensor_scalar_max(out=diff2, in0=psumD, scalar1=0.0)
            dist = epi.tile([16, 16], FP32)
            nc.scalar.activation(out=dist, in_=diff2,
                                 func=mybir.ActivationFunctionType.Sqrt)
            regk = epi.tile([16, 1], FP32)
            nc.scalar.activation(out=regk, in_=m2,
                                 func=mybir.ActivationFunctionType.Sqrt)

            hinge = epi.tile([16, 16], FP32)
            nc.vector.tensor_scalar(
                out=hinge, in0=dist, scalar1=-1.0, scalar2=2.0 * DD,
                op0=mybir.AluOpType.mult, op1=mybir.AluOpType.add,
            )
            nc.vector.tensor_scalar_max(out=hinge, in0=hinge, scalar1=0.0)
            nc.vector.tensor_mul(hinge, hinge, hinge)
            nc.vector.tensor_mul(hinge, hinge, sb_triu)

            final = epi.tile([16, 18], FP32)
            nc.vector.tensor_copy(final[:, 0:1], vark)
            nc.vector.tensor_copy(final[:, 1:2], regk)
            nc.vector.tensor_copy(final[:, 2:18], hinge)

            psumS = epsp.tile([1, 18], FP32)
            nc.tensor.matmul(psumS[:, :], sb_ones_col, final,
                             start=True, stop=True)
            fin = epi.tile([1, 18], FP32)
            nc.vector.tensor_copy(fin, psumS)

            hsum = epi.tile([1, 1], FP32)
            nc.vector.tensor_reduce(
                out=hsum, in_=fin[:, 2:18], axis=mybir.AxisListType.X,
                op=mybir.AluOpType.add,
            )
            gr = epi.tile([1, 1], FP32)
            nc.vector.tensor_scalar(
                out=gr, in0=fin[:, 1:2], scalar1=GAMMA, scalar2=None,
                op0=mybir.AluOpType.mult,
            )
            nc.vector.tensor_add(gr, gr, fin[:, 0:1])
            hh = epi.tile([1, 1], FP32)
            nc.vector.tensor_scalar(
                out=hh, in0=hsum, scalar1=1.0 / (K - 1), scalar2=None,
                op0=mybir.AluOpType.mult,
            )
            nc.vector.tensor_add(gr, gr, hh)
            loss = epi.tile([1, 1], FP32)
            nc.vector.tensor_scalar(
                out=loss, in0=gr, scalar1=1.0 / K, scalar2=None,
                op0=mybir.AluOpType.mult,
            )
            nc.sync.dma_start(out=out[:, :], in_=loss)

    _split_multi_waits(nc)
    return nc


_NC_CACHE = {}


def _get_kernel():
    key = (SQ_ACT_FRAC, SQ_SPLIT)
    if key not in _NC_CACHE:
        _NC_CACHE[key] = _build_kernel()
    return _NC_CACHE[key]


# --------------------------------------------------------------- entry point
def _marshal_image(feat: np.ndarray, lab: np.ndarray):
    # feat [C, H, W] f32 -> fpk [(ci t), (w g c)] fp16, pixel-on-partition
    f5 = feat.reshape(C, G, NCHUNK, CW, TW)
    fpk = np.ascontiguousarray(
        f5.transpose(2, 4, 3, 1, 0).reshape(NCHUNK * TW, CW * G * C)
    ).astype(np.float16)
    # lab [H, W] int -> labt [t, (w g)] fp16
    l3 = lab.reshape(G, NW, TW)
    labt = np.ascontiguousarray(
        l3.transpose(2, 1, 0).reshape(TW, NW * G)
    ).astype(np.float16)
    return fpk, labt


def kernel(features_batch, labels_batch, num_instances):
    assert int(num_instances) == K
    features_batch = np.asarray(features_batch, dtype=np.float32)
    labels_batch = np.asarray(labels_batch)
    assert features_batch.shape == (B, C, H, W)

    nc = _get_kernel()
    in_maps = []
    for i in range(B):
        fpk, labt = _marshal_image(features_batch[i], labels_batch[i])
        in_maps.append({"fpk": fpk, "labt": labt})

    res = run_bass_kernel_spmd(
        nc, in_maps, core_ids=list(range(B)), trace=TRACE
    )
    kernel.last_result = res
    losses = [res.results[i]["out"][0, 0] for i in range(B)]
    total = np.float64(0.0)
    for v in losses:
        total += np.float64(v)
    return np.array(total / (B + 1), dtype=np.float32)


# revision 3
# speedup vs baseline: 1.2473x; 1.2473x over previous
"""Trainium2 kernel for nn_ContrasiveLoss (segment-reduce contrastive loss).

Strategy (data-parallel, one image per NeuronCore, 8 cores):
  Per-image loss needs only per-segment statistics:
      counts[k], sums[k, c], s2[k] = sum_{n in k} ||f_n||^2
  (the variance term telescopes).  Statistics come from one-hot matmuls on
  the TensorEngine: pixels ride the contraction axis; a block-diagonal
  one-hot (8 pixel groups x 16 labels = 128 columns -> full-width stationary,
  which triggers the compiler's fast-weight-load) is the stationary operand.
  Features ship as fp8e4 (halves HBM traffic; tolerance is 2e-2), the
  one-hot stays fp16 (mixed-dtype matmul).  Two moving streams per window:
      A: features fp8e4 [128, 256]          -> psum_a [128, 256]
      B: [sq16 fp16 (128) | ones (1)]       -> psum_b [128, 129]
  where sq16 = squares pair-reduced 32ch -> 16ch.  Squares (fp8 in, fp16
  out) are split across ScalarE/GpSimd/VectorE; the pair-reduce ladder and
  one-hot run on VectorE at its 2x fp16 rate.  A tiny epilogue turns stats
  into the scalar loss; host sums the 8 scalars.
"""

import numpy as np
import ml_dtypes

import concourse.bass as bass
import concourse.mybir as mybir
import concourse.tile as tile
from concourse.bass_utils import run_bass_kernel_spmd
from concourse.vector_clock import ScopedClock

# ---------------------------------------------------------------- problem dims
B, C, H, W = 8, 32, 512, 512
K = 16
G = 8                    # pixel groups packed alongside channels (8*16 = 128)
N = H * W                # pixels per image
TW = 128                 # pixels (per group) contracted per matmul window
NW = N // (G * TW)       # 256 windows
CW = 16                  # windows per DMA chunk
NCHUNK = NW // CW        # 16 chunks
FCOLS = G * C            # 256 moving feature cols per window
BS = 130                 # per-window slot in the sq16 buffer (4B aligned)

DD = 2.5
GAMMA = 0.005

FP16 = mybir.dt.float16
FP32 = mybir.dt.float32
FP8E4 = mybir.dt.float8e4

TRACE = False            # test harness flips this for NTFF profiling
SQ_ACT_FRAC = 0.70       # fraction of the squares computed on ScalarE
SQ_POOL_FRAC = 0.30      # fraction of the squares computed on GpSimd

# ------------------------------------------------- container-specific patches
def _patch_tile_drain() -> None:
    """This container's walrus build accepts only ONE sync-wait command per
    instruction, but TileContext's tail drain attaches one wait per active
    semaphore lane.  Split the tail drain into a chain of single-wait drains.
    """
    if getattr(tile.TileContext, "_drain_split_patched", False):
        return

    def _drain_and_barrier(self, tick_clock, wait_clock):
        drain_inst = self.nc.sync.drain()
        wait_clock.add_sem_waits(
            drain_inst.ins, ScopedClock({None: tick_clock.global_clock})
        )
        si = drain_inst.ins.sync_info
        if si is not None and len(si.on_wait) > 1:
            waits = list(si.on_wait)
            drain_inst.ins.sync_info = mybir.SyncInfo(
                on_wait=[waits[0]], on_update=list(si.on_update)
            )
            for w in waits[1:]:
                d2 = self.nc.sync.drain()
                d2.ins.sync_info = mybir.SyncInfo(on_wait=[w], on_update=[])

        self.nc.all_engine_barrier()
        assert self.sems is not None
        popped = self.nc._tile_sem_poison_stack.pop()
        assert popped is self._sem_poison
        self.nc.clear_and_free_semaphores(list(self.sems.allocated().values()))
        self.nc.all_engine_barrier()

    tile.TileContext._drain_and_barrier = _drain_and_barrier
    tile.TileContext._drain_split_patched = True


def _split_multi_waits(nc) -> None:
    """Walrus accepts one sync-wait per instruction: hoist extra waits onto
    single-wait Drain instructions on the same engine, inserted just before."""
    for fn in nc.m.functions:
        for blk in fn.blocks:
            changed = False
            out = []
            for ins in blk.instructions:
                si = ins.sync_info
                if si is not None and len(si.on_wait) > 1:
                    changed = True
                    waits = list(si.on_wait)
                    for j, w in enumerate(waits[:-1]):
                        d = mybir.InstDrain(name=f"{ins.name}-ws{j}")
                        d.engine = ins.engine
                        d.sync_info = mybir.SyncInfo(on_wait=[w], on_update=[])
                        out.append(d)
                    ins.sync_info = mybir.SyncInfo(
                        on_wait=[waits[-1]], on_update=list(si.on_update)
                    )
                out.append(ins)
            if changed:
                blk.instructions = out


# ------------------------------------------------------------- device program
def _host_constants():
    # iota over one-hot columns j=(g,k): value = j % 16
    iota = np.broadcast_to(
        np.tile(np.arange(K, dtype=np.float16), G), (128, 128)
    ).copy()

    # stats layout: [A (256) | B (129)]; row r = (g, k)
    mask = np.zeros((128, FCOLS + 129), dtype=np.float32)
    for r in range(128):
        g = r // K
        mask[r, g * C:(g + 1) * C] = 1.0          # sums block of own group
        mask[r, FCOLS + g * 16:FCOLS + (g + 1) * 16] = 1.0  # sq16 block
        mask[r, FCOLS + 128] = 1.0                # counts column
    sel = np.zeros((128, K), dtype=np.float32)
    for r in range(128):
        sel[r, r % K] = 1.0
    ident16 = np.eye(16, dtype=np.float32)
    ones_row = np.ones((1, 16), dtype=np.float32)
    ones_col = np.ones((16, 1), dtype=np.float32)
    triu = np.triu(np.ones((K, K), dtype=np.float32), k=1)
    return iota, mask, sel, ident16, ones_row, ones_col, triu


def _build_kernel():
    _patch_tile_drain()
    nc = bass.Bass("TRN2")

    fpk = nc.dram_tensor("fpk", [NCHUNK * 128, CW * FCOLS], FP8E4,
                         kind="ExternalInput")
    labt = nc.dram_tensor("labt", [128, NW * G], FP16, kind="ExternalInput")
    out = nc.dram_tensor("out", [1, 1], FP32, kind="ExternalOutput")

    iota_np, mask_np, sel_np, id16_np, ones_row_np, ones_col_np, triu_np = \
        _host_constants()
    c_iota = nc.inline_tensor(iota_np, name="c_iota")
    c_mask = nc.inline_tensor(mask_np, name="c_mask")
    c_sel = nc.inline_tensor(sel_np, name="c_sel")
    c_id16 = nc.inline_tensor(id16_np, name="c_id16")
    c_ones_row = nc.inline_tensor(ones_row_np, name="c_ones_row")
    c_ones_col = nc.inline_tensor(ones_col_np, name="c_ones_col")
    c_triu = nc.inline_tensor(triu_np, name="c_triu")

    with tile.TileContext(nc) as tc:
        with (
            tc.tile_pool(name="consts", bufs=1) as consts,
            tc.tile_pool(name="oh", bufs=1) as ohp,
            tc.tile_pool(name="feat", bufs=3) as featp,
            tc.tile_pool(name="sq", bufs=3) as sqp,
            tc.tile_pool(name="s16", bufs=3) as s16p,
            tc.tile_pool(name="acc", bufs=1, space="PSUM") as accp,
            tc.tile_pool(name="eps", bufs=1, space="PSUM") as epsp,
            tc.tile_pool(name="epi", bufs=1) as epi,
        ):
            # ---- constants + labels into SBUF (scalar-engine DMA ring keeps
            # the sync ring free for the feature stream)
            sb_iota = consts.tile([128, 128], FP16)
            nc.scalar.dma_start(out=sb_iota, in_=c_iota[:, :])
            sb_mask = consts.tile([128, FCOLS + 129], FP32)
            nc.scalar.dma_start(out=sb_mask, in_=c_mask[:, :])
            sb_sel = consts.tile([128, K], FP32)
            nc.scalar.dma_start(out=sb_sel, in_=c_sel[:, :])
            sb_id16 = consts.tile([16, 16], FP32)
            nc.scalar.dma_start(out=sb_id16, in_=c_id16[:, :])
            sb_ones_row = consts.tile([1, 16], FP32)
            nc.scalar.dma_start(out=sb_ones_row, in_=c_ones_row[:, :])
            sb_ones_col = consts.tile([16, 1], FP32)
            nc.scalar.dma_start(out=sb_ones_col, in_=c_ones_col[:, :])
            sb_triu = consts.tile([16, 16], FP32)
            nc.scalar.dma_start(out=sb_triu, in_=c_triu[:, :])
            sb_labt = consts.tile([128, NW * G], FP16)
            nc.sync.dma_start(out=sb_labt, in_=labt[:, :])

            oh_full = ohp.tile([128, NW * 128], FP16)
            oh3 = oh_full.rearrange("p (w j) -> p w j", j=128)

            psum_a = accp.tile([128, FCOLS], FP32)
            psum_b = accp.tile([128, 129], FP32)

            ncols = CW * FCOLS
            pool_cols = int(ncols * SQ_POOL_FRAC) // 4 * 4
            act_cols = int(ncols * SQ_ACT_FRAC) // 4 * 4

            for ci in range(NCHUNK):
                # ---- feature chunk: straight contiguous 0.5MB DMA
                ft = featp.tile([128, CW * FCOLS], FP8E4)
                nc.sync.dma_start(
                    out=ft, in_=fpk[ci * 128:(ci + 1) * 128, :]
                )

                # ---- one-hot for this chunk's windows (DVE, fp16 2x)
                lab_b = bass.AP(
                    tensor=sb_labt.tensor,
                    offset=ci * CW * G,
                    ap=[[NW * G, 128], [G, CW], [1, G], [0, K]],
                )
                iota_b = bass.AP(
                    tensor=sb_iota.tensor,
                    offset=0,
                    ap=[[128, 128], [0, CW], [1, 128]],
                )
                nc.vector.tensor_tensor(
                    out=oh3[:, ci * CW:(ci + 1) * CW, :], in0=lab_b,
                    in1=iota_b, op=mybir.AluOpType.is_equal,
                )

                # ---- squares fp8 -> fp16 (POOL | ACT | DVE column split)
                sqf = sqp.tile([128, CW * FCOLS], FP16)
                if pool_cols > 0:
                    nc.gpsimd.tensor_mul(
                        sqf[:, 0:pool_cols],
                        ft[:, 0:pool_cols], ft[:, 0:pool_cols],
                    )
                a0, a1 = pool_cols, pool_cols + act_cols
                nc.scalar.activation(
                    out=sqf[:, a0:a1], in_=ft[:, a0:a1],
                    func=mybir.ActivationFunctionType.Square,
                )
                if a1 < ncols:
                    nc.vector.tensor_mul(
                        sqf[:, a1:ncols], ft[:, a1:ncols], ft[:, a1:ncols]
                    )

                # ---- pair-reduce 32ch -> 16ch (DVE fp16 2x) + ones column
                s16 = s16p.tile([128, CW, BS], FP16)
                sq4 = sqf.rearrange("p (w g c) -> p w g c", g=G, c=C)
                nc.vector.tensor_add(
                    s16[:, :, 0:128], sq4[:, :, :, 0:16], sq4[:, :, :, 16:32]
                )
                nc.vector.memset(s16[:, :, 128:129], 1.0)

                # ---- segment matmuls (one-hot stationary, 128 cols -> FWL)
                for wl in range(CW):
                    w = ci * CW + wl
                    lhsT = oh3[:, w, :]
                    nc.tensor.matmul(
                        psum_a[:, :], lhsT,
                        ft[:, wl * FCOLS:(wl + 1) * FCOLS],
                        start=(w == 0), stop=(w == NW - 1),
                    )
                    nc.tensor.matmul(
                        psum_b[:, :], lhsT, s16[:, wl, 0:129],
                        start=(w == 0), stop=(w == NW - 1),
                    )

            # ================= epilogue: stats -> scalar loss =================
            stats = epi.tile([128, FCOLS + 129], FP32)
            nc.vector.tensor_copy(stats[:, 0:FCOLS], psum_a)
            nc.vector.tensor_copy(stats[:, FCOLS:FCOLS + 129], psum_b)

            masked = epi.tile([128, FCOLS + 129], FP32)
            nc.vector.tensor_mul(masked, stats, sb_mask)

            psum2 = epsp.tile([16, FCOLS + 129], FP32)
            nc.tensor.matmul(psum2[:, :], sb_sel, masked, start=True, stop=True)
            comb = epi.tile([16, FCOLS + 129], FP32)
            nc.vector.tensor_copy(comb, psum2)

            # sums over the 8 groups: innermost-g view [16, c:32, g:8]
            sums = epi.tile([16, C], FP32)
            comb_gview = bass.AP(
                tensor=comb.tensor, offset=0,
                ap=[[FCOLS + 129, 16], [1, C], [C, G]],
            )
            nc.vector.tensor_reduce(
                out=sums, in_=comb_gview, axis=mybir.AxisListType.X,
                op=mybir.AluOpType.add,
            )
            s2 = epi.tile([16, 1], FP32)
            nc.vector.tensor_reduce(
                out=s2, in_=comb[:, FCOLS:FCOLS + 128],
                axis=mybir.AxisListType.X, op=mybir.AluOpType.add,
            )
            counts = epi.tile([16, 1], FP32)
            nc.vector.tensor_copy(counts, comb[:, FCOLS + 128:FCOLS + 129])
            recip = epi.tile([16, 1], FP32)
            nc.vector.reciprocal(out=recip, in_=counts)

            means = epi.tile([16, C], FP32)
            nc.vector.tensor_scalar_mul(out=means, in0=sums, scalar1=recip)
            msq = epi.tile([16, C], FP32)
            nc.vector.tensor_mul(msq, means, means)
            m2 = epi.tile([16, 1], FP32)
            nc.vector.tensor_reduce(
                out=m2, in_=msq, axis=mybir.AxisListType.X,
                op=mybir.AluOpType.add,
            )
            vark = epi.tile([16, 1], FP32)
            nc.vector.tensor_scalar_mul(out=vark, in0=s2, scalar1=recip)
            nc.vector.tensor_sub(vark, vark, m2)

            # pairwise distances: diff2 = m2_i + m2_j - 2 * means @ means.T
            psumT = epsp.tile([32, 16], FP32)
            nc.tensor.transpose(psumT[:, :], means, sb_id16)
            meansT = epi.tile([32, 16], FP32)
            nc.vector.tensor_copy(meansT, psumT)
            meansTn2 = epi.tile([32, 16], FP32)
            nc.vector.tensor_scalar_mul(out=meansTn2, in0=meansT, scalar1=-2.0)

            psumR = epsp.tile([1, 16], FP32)
            nc.tensor.transpose(psumR[:, :], m2, sb_id16)
            m2row = epi.tile([1, 16], FP32)
            nc.vector.tensor_copy(m2row, psumR)

            psumD = epsp.tile([16, 16], FP32)
            nc.tensor.matmul(psumD[:, :], sb_ones_row, m2row,
                             start=True, stop=False)
            nc.tensor.matmul(psumD[:, :], m2row, sb_ones_row,
                             start=False, stop=False)
            nc.tensor.matmul(psumD[:, :], meansTn2, meansT,
                             start=False, stop=True)

            diff2 = epi.tile([16, 16], FP32)
            nc.vector.tensor_scalar_max(out=diff2, in0=psumD, scalar1=0.0)
            dist = epi.tile([16, 16], FP32)
            nc.scalar.activation(out=dist, in_=diff2,
                                 func=mybir.ActivationFunctionType.Sqrt)
            regk = epi.tile([16, 1], FP32)
            nc.scalar.activation(out=regk, in_=m2,
                                 func=mybir.ActivationFunctionType.Sqrt)

            hinge = epi.tile([16, 16], FP32)
            nc.vector.tensor_scalar(
                out=hinge, in0=dist, scalar1=-1.0, scalar2=2.0 * DD,
                op0=mybir.AluOpType.mult, op1=mybir.AluOpType.add,
            )
            nc.vector.tensor_scalar_max(out=hinge, in0=hinge, scalar1=0.0)
            nc.vector.tensor_mul(hinge, hinge, hinge)
            nc.vector.tensor_mul(hinge, hinge, sb_triu)

            final = epi.tile([16, 18], FP32)
            nc.vector.tensor_copy(final[:, 0:1], vark)
            nc.vector.tensor_copy(final[:, 1:2], regk)
            nc.vector.tensor_copy(final[:, 2:18], hinge)

            psumS = epsp.tile([1, 18], FP32)
            nc.tensor.matmul(psumS[:, :], sb_ones_col, final,
                             start=True, stop=True)
            fin = epi.tile([1, 18], FP32)
            nc.vector.tensor_copy(fin, psumS)

            hsum = epi.tile([1, 1], FP32)
            nc.vector.tensor_reduce(
                out=hsum, in_=fin[:, 2:18], axis=mybir.AxisListType.X,
                op=mybir.AluOpType.add,
            )
            gr = epi.tile([1, 1], FP32)
            nc.vector.tensor_scalar(
                out=gr, in0=fin[:, 1:2], scalar1=GAMMA, scalar2=None,
                op0=mybir.AluOpType.mult,
            )
            nc.vector.tensor_add(gr, gr, fin[:, 0:1])
            hh = epi.tile([1, 1], FP32)
            nc.vector.tensor_scalar(
                out=hh, in0=hsum, scalar1=1.0 / (K - 1), scalar2=None,
                op0=mybir.AluOpType.mult,
            )
            nc.vector.tensor_add(gr, gr, hh)
            loss = epi.tile([1, 1], FP32)
            nc.vector.tensor_scalar(
                out=loss, in0=gr, scalar1=1.0 / K, scalar2=None,
                op0=mybir.AluOpType.mult,
            )
            nc.sync.dma_start(out=out[:, :], in_=loss)

    _split_multi_waits(nc)
    return nc


_NC_CACHE = {}


def _get_kernel():
    key = (SQ_ACT_FRAC, SQ_POOL_FRAC)
    if key not in _NC_CACHE:
        _NC_CACHE[key] = _build_kernel()
    return _NC_CACHE[key]


# --------------------------------------------------------------- entry point
def _marshal_image(feat: np.ndarray, lab: np.ndarray):
    # feat [C, H, W] f32 -> fpk [(ci t), (w g c)] fp8e4, pixel-on-partition
    f5 = feat.reshape(C, G, NCHUNK, CW, TW)
    fpk = np.ascontiguousarray(
        f5.transpose(2, 4, 3, 1, 0).reshape(NCHUNK * TW, CW * G * C)
    ).astype(ml_dtypes.float8_e4m3)
    # lab [H, W] int -> labt [t, (w g)] fp16
    l3 = lab.reshape(G, NW, TW)
    labt = np.ascontiguousarray(
        l3.transpose(2, 1, 0).reshape(TW, NW * G)
    ).astype(np.float16)
    return fpk, labt


def kernel(features_batch, labels_batch, num_instances):
    assert int(num_instances) == K
    features_batch = np.asarray(features_batch, dtype=np.float32)
    labels_batch = np.asarray(labels_batch)
    assert features_batch.shape == (B, C, H, W)

    nc = _get_kernel()
    in_maps = []
    for i in range(B):
        fpk, labt = _marshal_image(features_batch[i], labels_batch[i])
        in_maps.append({"fpk": fpk, "labt": labt})

    res = run_bass_kernel_spmd(
        nc, in_maps, core_ids=list(range(B)), trace=TRACE
    )
    kernel.last_result = res
    losses = [res.results[i]["out"][0, 0] for i in range(B)]
    total = np.float64(0.0)
    for v in losses:
        total += np.float64(v)
    return np.array(total / (B + 1), dtype=np.float32)
